# revision 8
# baseline (speedup 1.0000x reference)
"""Trainium2 Bass kernel for nn_DKWinners (per-segment argmax one-hot mask * x).

Reference semantics (per row of x[B, N], N = OUT_DIM*DPC):
  seg = x.reshape(B, OUT_DIM, DPC); idx = argmax(seg, -1)   # first max wins
  out = one_hot(idx) * seg

Algorithm per core (batch-sharded: 128 rows/core -> partition dim).
Per column tile of F elements (S = F/16 segments), 4 DVE passes:
  1. M = per-segment max            (native tensor_reduce over [128,S,16])
  2. t = (x >= M_b) ? (F - Idx) : 0 (custom DVE op; Idx = stream index, so
                                     F-Idx is a strictly-decreasing positive
                                     weight -> first max lane gets the
                                     largest weight in its segment)
  3. W = per-segment max of t       (native tensor_reduce)
  4. out = ((F - Idx) == W_b) ? x : 0  (custom DVE op; W uniquely identifies
                                     the first-argmax stream position, so this
                                     reproduces argmax's first-occurrence
                                     tie-break exactly)
All weights are integers <= F, exact in f32.
"""

import numpy as np

ROWS = 1024
N = 65536
DPC = 16
N_CORES = 8
ROWS_PER_CORE = ROWS // N_CORES  # 128 -> partition dim

F = 4096          # free-dim tile size (per partition)
S = F // DPC      # segments per tile

_cache = {}
_dve_ops = {}


def _register_dve_ops():
    """Define + register the two custom DVE ops (idempotent)."""
    if _dve_ops:
        return _dve_ops

    from concourse import dve_ops
    from concourse.dve_spec import (
        C0,
        Idx,
        Spec,
        Src0,
        Src1,
        Zero,
        eq as dve_eq,
        lower,
        select,
    )
    from concourse.dve_table_gen import dve_ver_for, free_opcode_rows
    from concourse.dve_uop import DveOpSpec

    def _ref_weight(in0, in1, c0, c1, c2):
        p = in0.shape[0]
        x = np.asarray(in0, np.float32).reshape(p, -1)
        m = np.asarray(in1, np.float32).reshape(p, -1)
        idx = np.arange(x.shape[1], dtype=np.float32)[None, :]
        return np.where(x >= m, c0 - idx, 0.0).astype(np.float32)

    def _ref_pick(in0, in1, c0, c1, c2):
        p = in0.shape[0]
        x = np.asarray(in0, np.float32).reshape(p, -1)
        w = np.asarray(in1, np.float32).reshape(p, -1)
        idx = np.arange(x.shape[1], dtype=np.float32)[None, :]
        return np.where((c0 - idx) == w, x, 0.0).astype(np.float32)

    specs = {
        "SEG_ARGMAX_WEIGHT_ANT": Spec(
            body=select(Src0 >= Src1, C0 - Idx, Zero), reference=_ref_weight
        ),
        "SEG_ARGMAX_PICK_ANT": Spec(
            body=select(dve_eq(C0 - Idx, Src1), Src0, Zero), reference=_ref_pick
        ),
    }

    next_row = max(dve_ops._SUB_OPCODE_FOR_NAME.values()) + 1
    free_rows = set(free_opcode_rows("TRN2"))
    for name, spec in specs.items():
        if name in dve_ops._SUB_OPCODE_FOR_NAME:
            _dve_ops[name] = next(o for o in dve_ops.OPS if o.name == name)
            continue
        row = next_row
        next_row += 1
        assert row in free_rows, (row, sorted(free_rows))
        # compute the uops sha for every ver so DveOp.compile's pin check passes
        shas = {}
        for ver in ("v3", "v4"):
            try:
                uops = lower(spec, ver=ver)
            except Exception:
                continue
            shas[ver] = DveOpSpec(
                name=name, opcode=row, uops=uops, rd1_en=True
            ).sha(ver)
        op = dve_ops.DveOp(name, spec, subdim=False, uops_sha=shas)
        dve_ops._SUB_OPCODE_FOR_NAME[name] = row
        dve_ops.OPS.append(op)
        dve_ops.CUSTOM_DVE_SPECS[name] = spec
        _dve_ops[name] = op
    return _dve_ops


def _build_nc(n_cols):
    from contextlib import ExitStack

    import concourse.tile as tile
    from concourse import bacc, mybir

    ops = _register_dve_ops()
    w_op = ops["SEG_ARGMAX_WEIGHT_ANT"]
    p_op = ops["SEG_ARGMAX_PICK_ANT"]

    dt = mybir.dt
    alu = mybir.AluOpType

    nc = bacc.Bacc(
        "TRN2",
        target_bir_lowering=False,
        debug=False,
        enable_asserts=False,
    )
    x = nc.dram_tensor("x", [128, n_cols], dt.float32, kind="ExternalInput").ap()
    out = nc.dram_tensor("out", [128, n_cols], dt.float32, kind="ExternalOutput").ap()

    nt = n_cols // F
    assert n_cols % F == 0
    s = F // DPC

    with tile.TileContext(nc) as tc, ExitStack() as ctx:
        xp = ctx.enter_context(tc.tile_pool(name="xt", bufs=4))
        tp = ctx.enter_context(tc.tile_pool(name="tt", bufs=3))
        mp = ctx.enter_context(tc.tile_pool(name="mt", bufs=2))
        wp = ctx.enter_context(tc.tile_pool(name="wt", bufs=2))

        for i in range(nt):
            xt = xp.tile([128, F], dt.float32)
            nc.sync.dma_start(xt[:], x[:, i * F : (i + 1) * F])
            xv = xt[:].rearrange("p (s l) -> p s l", l=DPC)

            mt = mp.tile([128, s], dt.float32)
            m3 = mt[:].rearrange("p (s o) -> p s o", o=1)
            nc.vector.tensor_reduce(m3, xv, axis=mybir.AxisListType.X, op=alu.max)

            tt = tp.tile([128, F], dt.float32)
            nc.vector._custom_dve(
                w_op,
                out=tt[:],
                in0=xt[:],
                in1=m3.broadcast_to((128, s, DPC)),
                s0=float(F),
            )

            wt = wp.tile([128, s], dt.float32)
            w3 = wt[:].rearrange("p (s o) -> p s o", o=1)
            tv = tt[:].rearrange("p (s l) -> p s l", l=DPC)
            nc.vector.tensor_reduce(w3, tv, axis=mybir.AxisListType.X, op=alu.max)

            # write the final masked output into the (now dead) t tile so the
            # x tile frees as soon as this op has read it
            nc.vector._custom_dve(
                p_op,
                out=tt[:],
                in0=xt[:],
                in1=w3.broadcast_to((128, s, DPC)),
                s0=float(F),
            )
            nc.sync.dma_start(out[:, i * F : (i + 1) * F], tt[:])

    nc.compile()
    return nc


def _get_nc(n_cols=N):
    if n_cols not in _cache:
        _cache[n_cols] = _build_nc(n_cols)
    return _cache[n_cols]


def kernel(x):
    from concourse import bass_utils

    x = np.ascontiguousarray(x, dtype=np.float32)
    assert x.shape == (ROWS, N), x.shape
    nc = _get_nc(N)
    in_maps = [
        {"x": x[i * ROWS_PER_CORE : (i + 1) * ROWS_PER_CORE]} for i in range(N_CORES)
    ]
    res = bass_utils.run_bass_kernel_spmd(nc, in_maps, core_ids=list(range(N_CORES)))
    return np.concatenate([r["out"] for r in res.results], axis=0)


# revision 9
# speedup vs baseline: 1.0280x; 1.0280x over previous
"""Trainium2 Bass kernel for nn_DKWinners (per-segment argmax one-hot mask * x).

Reference semantics (per row of x[B, N], N = OUT_DIM*DPC):
  seg = x.reshape(B, OUT_DIM, DPC); idx = argmax(seg, -1)   # first max wins
  out = one_hot(idx) * seg

Algorithm per core (batch-sharded: 128 rows/core -> partition dim).
Per column tile of F elements (S = F/16 segments), 4 DVE passes:
  1. M = per-segment max            (native tensor_reduce over [128,S,16])
  2. t = (x >= M_b) ? (F - Idx) : 0 (custom DVE op; Idx = stream index, so
                                     F-Idx is a strictly-decreasing positive
                                     weight -> first max lane gets the
                                     largest weight in its segment)
  3. W = per-segment max of t       (native tensor_reduce)
  4. out = ((F - Idx) == W_b) ? x : 0  (custom DVE op; W uniquely identifies
                                     the first-argmax stream position, so this
                                     reproduces argmax's first-occurrence
                                     tie-break exactly)
All weights are integers <= F, exact in f32.
"""

import numpy as np

ROWS = 1024
N = 65536
DPC = 16
N_CORES = 8
ROWS_PER_CORE = ROWS // N_CORES  # 128 -> partition dim

F = 4096          # free-dim tile size (per partition)
S = F // DPC      # segments per tile

_cache = {}
_dve_ops = {}


def _register_dve_ops():
    """Define + register the two custom DVE ops (idempotent)."""
    if _dve_ops:
        return _dve_ops

    from concourse import dve_ops
    from concourse.dve_spec import (
        C0,
        Idx,
        Spec,
        Src0,
        Src1,
        Zero,
        eq as dve_eq,
        lower,
        select,
    )
    from concourse.dve_table_gen import dve_ver_for, free_opcode_rows
    from concourse.dve_uop import DveOpSpec

    def _ref_weight(in0, in1, c0, c1, c2):
        p = in0.shape[0]
        x = np.asarray(in0, np.float32).reshape(p, -1)
        m = np.asarray(in1, np.float32).reshape(p, -1)
        idx = np.arange(x.shape[1], dtype=np.float32)[None, :]
        return np.where(x >= m, c0 - idx, 0.0).astype(np.float32)

    def _ref_pick(in0, in1, c0, c1, c2):
        p = in0.shape[0]
        x = np.asarray(in0, np.float32).reshape(p, -1)
        w = np.asarray(in1, np.float32).reshape(p, -1)
        idx = np.arange(x.shape[1], dtype=np.float32)[None, :]
        return np.where((c0 - idx) == w, x, 0.0).astype(np.float32)

    specs = {
        "SEG_ARGMAX_WEIGHT_ANT": Spec(
            body=select(Src0 >= Src1, C0 - Idx, Zero), reference=_ref_weight
        ),
        "SEG_ARGMAX_PICK_ANT": Spec(
            body=select(dve_eq(C0 - Idx, Src1), Src0, Zero), reference=_ref_pick
        ),
    }

    next_row = max(dve_ops._SUB_OPCODE_FOR_NAME.values()) + 1
    free_rows = set(free_opcode_rows("TRN2"))
    for name, spec in specs.items():
        if name in dve_ops._SUB_OPCODE_FOR_NAME:
            _dve_ops[name] = next(o for o in dve_ops.OPS if o.name == name)
            continue
        row = next_row
        next_row += 1
        assert row in free_rows, (row, sorted(free_rows))
        # compute the uops sha for every ver so DveOp.compile's pin check passes
        shas = {}
        for ver in ("v3", "v4"):
            try:
                uops = lower(spec, ver=ver)
            except Exception:
                continue
            shas[ver] = DveOpSpec(
                name=name, opcode=row, uops=uops, rd1_en=True
            ).sha(ver)
        op = dve_ops.DveOp(name, spec, subdim=False, uops_sha=shas)
        dve_ops._SUB_OPCODE_FOR_NAME[name] = row
        dve_ops.OPS.append(op)
        dve_ops.CUSTOM_DVE_SPECS[name] = spec
        _dve_ops[name] = op
    return _dve_ops


def _build_nc(n_cols):
    from contextlib import ExitStack

    import concourse.tile as tile
    from concourse import bacc, mybir

    ops = _register_dve_ops()
    w_op = ops["SEG_ARGMAX_WEIGHT_ANT"]
    p_op = ops["SEG_ARGMAX_PICK_ANT"]

    dt = mybir.dt
    alu = mybir.AluOpType

    nc = bacc.Bacc(
        "TRN2",
        target_bir_lowering=False,
        debug=False,
        enable_asserts=False,
    )
    x = nc.dram_tensor("x", [128, n_cols], dt.float32, kind="ExternalInput").ap()
    out = nc.dram_tensor("out", [128, n_cols], dt.float32, kind="ExternalOutput").ap()

    # tapered schedule: half-size tiles at both ends shorten pipeline
    # fill (first load) and drain (last store); full F tiles in the middle
    half = F // 2
    assert n_cols % F == 0 and n_cols >= 2 * F
    sizes = [half, half] + [F] * ((n_cols - 2 * F) // F) + [half, half]
    assert sum(sizes) == n_cols

    with tile.TileContext(nc) as tc, ExitStack() as ctx:
        xp = ctx.enter_context(tc.tile_pool(name="xt", bufs=4))
        tp = ctx.enter_context(tc.tile_pool(name="tt", bufs=3))
        mp = ctx.enter_context(tc.tile_pool(name="mt", bufs=2))
        wp = ctx.enter_context(tc.tile_pool(name="wt", bufs=2))

        off = 0
        for fi in sizes:
            s = fi // DPC
            xt = xp.tile([128, fi], dt.float32, tag="xt")
            nc.sync.dma_start(xt[:], x[:, off : off + fi])
            xv = xt[:].rearrange("p (s l) -> p s l", l=DPC)

            mt = mp.tile([128, s], dt.float32, tag="mt")
            m3 = mt[:].rearrange("p (s o) -> p s o", o=1)
            nc.vector.tensor_reduce(m3, xv, axis=mybir.AxisListType.X, op=alu.max)

            tt = tp.tile([128, fi], dt.float32, tag="tt")
            nc.vector._custom_dve(
                w_op,
                out=tt[:],
                in0=xt[:],
                in1=m3.broadcast_to((128, s, DPC)),
                s0=float(fi),
            )

            wt = wp.tile([128, s], dt.float32, tag="wt")
            w3 = wt[:].rearrange("p (s o) -> p s o", o=1)
            tv = tt[:].rearrange("p (s l) -> p s l", l=DPC)
            nc.vector.tensor_reduce(w3, tv, axis=mybir.AxisListType.X, op=alu.max)

            # write the final masked output into the (now dead) t tile so the
            # x tile frees as soon as this op has read it
            nc.vector._custom_dve(
                p_op,
                out=tt[:],
                in0=xt[:],
                in1=w3.broadcast_to((128, s, DPC)),
                s0=float(fi),
            )
            nc.sync.dma_start(out[:, off : off + fi], tt[:])
            off += fi

    nc.compile()
    return nc


def _get_nc(n_cols=N):
    if n_cols not in _cache:
        _cache[n_cols] = _build_nc(n_cols)
    return _cache[n_cols]


def kernel(x):
    from concourse import bass_utils

    x = np.ascontiguousarray(x, dtype=np.float32)
    assert x.shape == (ROWS, N), x.shape
    nc = _get_nc(N)
    in_maps = [
        {"x": x[i * ROWS_PER_CORE : (i + 1) * ROWS_PER_CORE]} for i in range(N_CORES)
    ]
    res = bass_utils.run_bass_kernel_spmd(nc, in_maps, core_ids=list(range(N_CORES)))
    return np.concatenate([r["out"] for r in res.results], axis=0)
